# revision 61
# baseline (speedup 1.0000x reference)
"""Cosformer attention Bass kernel for 8 trn2 NeuronCores.

Sharding: core c handles batch c//2, sequence half c%2 (1024 tokens).
Per-head linear-attention state (kv, ksum) is AllReduce'd (bf16) between
the two cores sharing a batch.

v3: feature-major q projection fed directly from x_fm8 with LN1 folded
in as a rank-1 (mu x colsum(Wq)) correction + rstd folded into the s/c
position tensors -- this removes the qn DRAM round-trip/transposes and
makes the q GEMM independent of LN1, so it hides the kv AllReduce.
Wo+I is folded host-side (removes the final residual pass). LN stats
run on ACT (Square+accum_out); epilogues are split across DVE/ACT/
GPSIMD so the PE never starves and HAM stays warm.
"""

import sys

for _p in ('/opt/trn_rl_repo',):
    if _p not in sys.path:
        sys.path.insert(0, _p)

import importlib.util as _ilu
import os

os.environ.setdefault('NEURON_RT_RESET_CORES', '1')


# The image's antenv may lack axon_hooks (needed for trace=True); register ours.
def _ensure_axon_hooks():
    try:
        import antenv.axon_hooks  # noqa: F401
        return
    except ImportError:
        pass
    for _hp in ('/opt/trn_rl_repo/antenv/axon_hooks.py',):
        if os.path.exists(_hp):
            _spec = _ilu.spec_from_file_location('antenv.axon_hooks', _hp)
            _mod = _ilu.module_from_spec(_spec)
            _spec.loader.exec_module(_mod)
            sys.modules['antenv.axon_hooks'] = _mod
            return
    import types
    _mod = types.ModuleType('antenv.axon_hooks')
    _mod._hook = None
    def _set(h):
        _mod._hook = h
    def _get():
        return _mod._hook
    _mod.set_axon_ntff_profile_hook = _set
    _mod.get_axon_ntff_profile_hook = _get
    sys.modules['antenv.axon_hooks'] = _mod
    try:
        from trn_agent_boot.trn_boot import _ntff_profile_via_ctypes
        _h = _ntff_profile_via_ctypes('/opt/axon/libaxon_pjrt.so')
        if _h is not None:
            _mod._hook = _h
    except Exception:
        pass


_ensure_axon_hooks()

import numpy as np
import ml_dtypes

import concourse.bass as bass
import concourse.tile as tile
from concourse import bacc, mybir
from concourse.alu_op_type import AluOpType
from concourse.bass_utils import run_bass_kernel_spmd

BF16 = ml_dtypes.bfloat16
F8 = ml_dtypes.float8_e4m3
FP32 = mybir.dt.float32
BF = mybir.dt.bfloat16
F8E4 = mybir.dt.float8e4
AF = mybir.ActivationFunctionType
DR = mybir.MatmulPerfMode.DoubleRow

L, N, E, H, D = 2048, 4, 1024, 16, 64
T = 1024            # tokens per core
NT = T // 128       # 8 token tiles
NK = E // 128       # 8 contraction tiles
NK2 = NK // 2       # 4 DoubleRow pair steps
NJ = E // 128       # 8 output-feature tiles
NCORES = 8
EPS_LN = 1e-5

EW = 8              # weight scale exponent: W8 = W * 2^EW
SKV = 2.0 ** (2 - EW)   # k/v epilogue scale (psum -> activation)
SO = 2.0 ** (-EW)       # o epilogue scale
SQH = 2.0 ** (-3)       # host fold into s/c tensors: q ends up x 2^5

_BUILD_CACHE = {}


def _build_program():
    nc = bacc.Bacc("TRN2", target_bir_lowering=False, debug=False,
                   num_devices=NCORES)

    # ---- DRAM I/O ----
    d_x_tm = nc.dram_tensor('x_tm', [T, E], BF, kind='ExternalInput')
    d_x_fm = nc.dram_tensor('x_fm', [E, T], F8E4, kind='ExternalInput')
    d_wq = nc.dram_tensor('wq', [E, E], F8E4, kind='ExternalInput')
    d_wk = nc.dram_tensor('wk', [E, E], F8E4, kind='ExternalInput')
    d_wv = nc.dram_tensor('wv', [E, E], F8E4, kind='ExternalInput')
    d_wo = nc.dram_tensor('wo', [E, E], F8E4, kind='ExternalInput')
    d_sb = nc.dram_tensor('s_bcast', [128, T], BF, kind='ExternalInput')
    d_cb = nc.dram_tensor('c_bcast', [128, T], BF, kind='ExternalInput')
    d_scol = nc.dram_tensor('s_cols', [128, NT], FP32, kind='ExternalInput')
    d_ccol = nc.dram_tensor('c_cols', [128, NT], FP32, kind='ExternalInput')
    d_csw = nc.dram_tensor('neg_csw', [1, E], BF, kind='ExternalInput')
    d_out = nc.dram_tensor('out', [T, E], FP32, kind='ExternalOutput')

    RG = [[0, 1], [2, 3], [4, 5], [6, 7]]
    INV_E = 1.0 / float(E)

    with tile.TileContext(nc) as tc:
        with (
            tc.tile_pool(name='persist', bufs=1) as pp,
            tc.tile_pool(name='dram', bufs=1, space='DRAM') as dp,
        ):
            # ---- persistent tiles ----
            sbt = pp.tile([128, T], BF, tag='sbt')
            cbt = pp.tile([128, T], BF, tag='cbt')
            scol = pp.tile([128, NT], FP32, tag='scol')
            ccol = pp.tile([128, NT], FP32, tag='ccol')
            cswt = pp.tile([1, E], BF, tag='cswt')
            eps1 = pp.tile([128, 1], FP32, tag='eps1')
            wk8 = pp.tile([128, NK, E], F8E4, tag='wk8')
            wv8 = pp.tile([128, NK, E], F8E4, tag='wv8')
            wq8 = pp.tile([128, NK, E], F8E4, tag='wq8')
            wo8 = pp.tile([128, NK, E], F8E4, tag='wo8')
            xfm8 = pp.tile([128, NK, T], F8E4, tag='xfm8')
            ksc8 = pp.tile([128, NT, H, 128], F8E4, tag='ksc8')
            vaug8 = pp.tile([128, NT, H, 65], F8E4, tag='vaug8')
            qn_sb = pp.tile([128, NT, E], BF, tag='qn_sb')
            qq8 = pp.tile([128, H, T], BF, tag='qq8')
            kvp = pp.tile([128, H * 65], BF, tag='kvp')
            yt_all = pp.tile([128, NT, H, 64], BF, tag='yt_all')
            xh_sb = pp.tile([128, NT, E], BF, tag='xh_sb')
            xhT8 = pp.tile([128, NK, T], F8E4, tag='xhT8')
            wz = pp.tile([128, 512], BF, tag='wz')
            ascr = pp.tile([128, E], BF, tag='ascr')       # ACT accum scratch

            # DRAM scratch
            xh_dram = dp.tile([T, E], BF)
            mrs_dram = dp.tile([T, 128], BF)
            kv_cc_in1 = dp.tile([128, 8 * 65], BF)
            kv_cc_out1 = dp.tile([128, 8 * 65], BF)
            kv_cc_in2 = dp.tile([128, 8 * 65], BF)
            kv_cc_out2 = dp.tile([128, 8 * 65], BF)

            # ---- PE warm-up: dense matmuls on a zeroed tile ----
            nc.vector.memset(wz, 0.0)
            nc.vector.memset(eps1, EPS_LN)
            # ones column carries the NET v scale (2^EW * SKV = 2^2)
            nc.gpsimd.memset(vaug8[:, :, :, 64:65], 2.0 ** 2)
            with tc.tile_pool(name='psW', bufs=1, space='PSUM') as psw:
                pw = psw.tile([128, 512], FP32, tag='psW')
                for _ in range(20):
                    nc.tensor.matmul(pw, lhsT=wz[:, 0:128], rhs=wz,
                                     start=True, stop=True)

            # ---- input DMAs (order per queue == priority) ----
            xfm_src = d_x_fm[:].rearrange('(k p) t -> p k t', p=128)
            wk_src = d_wk[:].rearrange('(k p) e -> p k e', p=128)
            wv_src = d_wv[:].rearrange('(k p) e -> p k e', p=128)
            wq_src = d_wq[:].rearrange('(k p) e -> p k e', p=128)
            wo_src = d_wo[:].rearrange('(k p) e -> p k e', p=128)
            for k in range(NK):
                nc.sync.dma_start(out=xfm8[:, k, :], in_=xfm_src[:, k, :])
                nc.scalar.dma_start(out=wk8[:, k, :], in_=wk_src[:, k, :])
                nc.gpsimd.dma_start(out=wv8[:, k, :], in_=wv_src[:, k, :])
            nc.sync.dma_start(out=scol, in_=d_scol[:])
            nc.sync.dma_start(out=ccol, in_=d_ccol[:])
            nc.sync.dma_start(out=sbt, in_=d_sb[:])
            nc.sync.dma_start(out=cbt, in_=d_cb[:])
            nc.sync.dma_start(out=cswt, in_=d_csw[:])
            for k in range(NK):
                nc.gpsimd.dma_start(out=wq8[:, k, :], in_=wq_src[:, k, :])
            for k in range(NK):
                nc.gpsimd.dma_start(out=wo8[:, k, :], in_=wo_src[:, k, :])

            # ================= A phase: LN1 stats + qn (token-major) ========
            # stats via ACT Square/Copy + accum_out; qn via GPSIMD
            with tc.tile_pool(name='ln1', bufs=3) as ap:
                for i in range(NT):
                    xt = ap.tile([128, E], BF, tag='xt')
                    nc.sync.dma_start(out=xt,
                                      in_=d_x_tm[i * 128:(i + 1) * 128, :])
                    st = ap.tile([128, 2, 6], FP32, tag='st')
                    xg = xt[:].rearrange('p (g d) -> p g d', g=2)
                    nc.vector.bn_stats(out=st[:, 0, :], in_=xg[:, 0, :])
                    nc.vector.bn_stats(out=st[:, 1, :], in_=xg[:, 1, :])
                    mv = ap.tile([128, 2], FP32, tag='mv')
                    nc.vector.bn_aggr(out=mv, in_=st)
                    sd = ap.tile([128, 1], FP32, tag='sd')
                    nc.scalar.activation(out=sd, in_=mv[:, 1:2], func=AF.Sqrt,
                                         bias=eps1, scale=1.0)
                    rstd = ap.tile([128, 1], FP32, tag='rstd')
                    nc.vector.reciprocal(out=rstd, in_=sd)
                    mub = ap.tile([128, 1], BF, tag='mub')
                    nc.vector.tensor_copy(out=mub, in_=mv[:, 0:1])
                    nc.sync.dma_start(
                        out=mrs_dram[i * 128:(i + 1) * 128, 0:1], in_=mub)
                    nc.vector.tensor_scalar(out=qn_sb[:, i, :], in0=xt,
                                            scalar1=mv[:, 0:1], scalar2=rstd,
                                            op0=AluOpType.subtract,
                                            op1=AluOpType.mult)
            # mu row (partition 0) for the rank-1 LN1 fold in the q GEMM.
            # rstd needs no handling at all: a per-token scale on q cancels
            # in the attention num/den ratio, and relu commutes with it.
            bcT = pp.tile([128, T], BF, tag='bcT')
            nc.sync.dma_start(out=bcT, in_=mrs_dram[:], transpose=True)

            # ================= B phase: fused k/v proj + kv accumulation ====
            with (
                tc.tile_pool(name='psB', bufs=2, space='PSUM') as psb,
                tc.tile_pool(name='psKV', bufs=4, space='PSUM') as pskv,
            ):
                kvt = [pskv.tile([128, 4, 65], FP32, tag='psKV',
                                 name=f'kvt_{g}') for g in range(4)]

                for i in range(NT):
                    pk = psb.tile([128, 1024], FP32, tag='psB', name=f'pk_{i}')
                    pv = psb.tile([128, 1024], FP32, tag='psB', name=f'pv_{i}')
                    lhs = lambda k2: xfm8[:, 2 * k2:2 * k2 + 2,
                                          i * 128:(i + 1) * 128]
                    for k2 in range(NK2):
                        for ch in range(2):
                            csl = slice(ch * 512, (ch + 1) * 512)
                            nc.tensor.matmul(
                                pk[:, csl], lhsT=lhs(k2),
                                rhs=wk8[:, 2 * k2:2 * k2 + 2, csl],
                                start=(k2 == 0), stop=(k2 == NK2 - 1),
                                perf_mode=DR)
                    for k2 in range(NK2):
                        for ch in range(2):
                            csl = slice(ch * 512, (ch + 1) * 512)
                            nc.tensor.matmul(
                                pv[:, csl], lhsT=lhs(k2),
                                rhs=wv8[:, 2 * k2:2 * k2 + 2, csl],
                                start=(k2 == 0), stop=(k2 == NK2 - 1),
                                perf_mode=DR)
                    # k epilogue on ACT: relu(scol*pk) == scol*relu(pk)
                    # (scol, ccol >= 0)
                    for ch in range(2):
                        csl = slice(ch * 512, (ch + 1) * 512)
                        pkv = pk[:, csl].rearrange('p (h d) -> p h d', d=64)
                        nc.scalar.activation(
                            out=ksc8[:, i, ch * 8:(ch + 1) * 8, 0:64],
                            in_=pkv, func=AF.Relu, scale=scol[:, i:i + 1])
                        nc.scalar.activation(
                            out=ksc8[:, i, ch * 8:(ch + 1) * 8, 64:128],
                            in_=pkv, func=AF.Relu, scale=ccol[:, i:i + 1])
                        nc.scalar.activation(
                            out=vaug8[:, i, ch * 8:(ch + 1) * 8, 0:64],
                            in_=pv[:, csl].rearrange('p (h d) -> p h d', d=64),
                            func=AF.Copy, scale=SKV)
                # kv accumulation: sequential per-head groups, split into
                # two halves so the first AllReduce can trigger early.
                # PSUM->SBUF copies on ACT (must not queue behind DVE work).
                kvv = kvp[:].rearrange('p (h c) -> p h c', c=65)
                for h in range(8):
                    for i in range(NT):
                        nc.tensor.matmul(kvt[h // 4][:, h % 4, :],
                                         lhsT=ksc8[:, i, h, :],
                                         rhs=vaug8[:, i, h, :],
                                         start=(i == 0), stop=(i == NT - 1))
                for g in range(2):
                    nc.scalar.activation(out=kvv[:, g * 4:(g + 1) * 4, :],
                                         in_=kvt[g], func=AF.Copy)
                nc.gpsimd.dma_start(out=kv_cc_in1[:], in_=kvp[:, 0:520])
                nc.gpsimd.collective_compute(
                    'AllReduce', AluOpType.add,
                    ins=[kv_cc_in1.opt()], outs=[kv_cc_out1.opt()],
                    replica_groups=RG)
                for h in range(8, H):
                    for i in range(NT):
                        nc.tensor.matmul(kvt[h // 4][:, h % 4, :],
                                         lhsT=ksc8[:, i, h, :],
                                         rhs=vaug8[:, i, h, :],
                                         start=(i == 0), stop=(i == NT - 1))
                for g in range(2, 4):
                    nc.scalar.activation(out=kvv[:, g * 4:(g + 1) * 4, :],
                                         in_=kvt[g], func=AF.Copy)
                nc.gpsimd.dma_start(out=kv_cc_in2[:], in_=kvp[:, 520:1040])
                nc.gpsimd.collective_compute(
                    'AllReduce', AluOpType.add,
                    ins=[kv_cc_in2.opt()], outs=[kv_cc_out2.opt()],
                    replica_groups=RG)

            kvb_bf = pp.tile([128, H * 65], BF, tag='kvb_bf')
            nc.gpsimd.dma_start(out=kvb_bf[:, 0:520], in_=kv_cc_out1[:])
            nc.gpsimd.dma_start(out=kvb_bf[:, 520:1040], in_=kv_cc_out2[:])

            # ================= D phase: q projection (feature-major) ========
            # pq[j,t] = (x @ Wq)[j,t] * 2^8 ; LN1 folds in via
            # t1 = pq + mu_b * (-colsum(Wq) * 2^8), then relu, then
            # qq8 = relu(t1) * (s|c * rstd * 2^-3)
            with (
                tc.tile_pool(name='psD', bufs=3, space='PSUM') as psd,
                tc.tile_pool(name='qsb', bufs=3) as qsb,
            ):
                for j in range(NJ):
                    pq = psd.tile([128, 1024], FP32, tag='psD',
                                  name=f'pq_{j}')
                    for k2 in range(NK2):
                        for ch in range(2):
                            csl = slice(ch * 512, (ch + 1) * 512)
                            nc.tensor.matmul(
                                pq[:, csl],
                                lhsT=wq8[:, 2 * k2:2 * k2 + 2,
                                         j * 128:(j + 1) * 128],
                                rhs=xfm8[:, 2 * k2:2 * k2 + 2, csl],
                                start=(k2 == 0), stop=False,
                                perf_mode=DR)
                    # rank-1 LN1 fold: pq += (-colsum(Wq) row)^T @ mu row
                    for ch in range(2):
                        csl = slice(ch * 512, (ch + 1) * 512)
                        nc.tensor.matmul(
                            pq[:, csl],
                            lhsT=cswt[0:1, j * 128:(j + 1) * 128],
                            rhs=bcT[0:1, csl], start=False, stop=True)
                    qrel = qsb.tile([128, 1024], BF, tag='qrel')
                    nc.scalar.activation(out=qrel, in_=pq, func=AF.Relu)
                    h0, h1 = 2 * j, 2 * j + 1
                    nc.vector.tensor_tensor(
                        out=qq8[0:64, h0, :], in0=qrel[0:64, :],
                        in1=sbt[0:64, :], op=AluOpType.mult)
                    nc.vector.tensor_tensor(
                        out=qq8[64:128, h0, :], in0=qrel[0:64, :],
                        in1=cbt[0:64, :], op=AluOpType.mult)
                    nc.vector.tensor_tensor(
                        out=qq8[0:64, h1, :], in0=qrel[64:128, :],
                        in1=sbt[64:128, :], op=AluOpType.mult)
                    nc.vector.tensor_tensor(
                        out=qq8[64:128, h1, :], in0=qrel[64:128, :],
                        in1=cbt[64:128, :], op=AluOpType.mult)

            # ================= E phase: attention + LN2 (token-major) =======
            xhT = pp.tile([128, NJ, 512], BF, tag='xhT')

            def emit_xh_transpose(tsl, par):
                for j in range(NJ):
                    eng = nc.sync if j % 2 == par else nc.scalar
                    eng.dma_start(out=xhT[:, j, :],
                                  in_=xh_dram[tsl, j * 128:(j + 1) * 128],
                                  transpose=True)
                    nc.scalar.activation(out=xhT8[:, j, tsl],
                                         in_=xhT[:, j, :], func=AF.Copy)

            with (
                tc.tile_pool(name='psE', bufs=6, space='PSUM') as pse,
                tc.tile_pool(name='ef', bufs=3) as efp,
                tc.tile_pool(name='psG', bufs=2, space='PSUM') as psg,
                tc.tile_pool(name='go', bufs=3) as gop,
            ):
                def emit_attn_pass(i, p):
                    rsl = slice(i * 128, (i + 1) * 128)
                    z8 = efp.tile([128, 8], FP32, tag='z8',
                                  name=f'z8_{i}_{p}')
                    pas = []
                    for gg in range(2):
                        g = 2 * p + gg
                        pa = pse.tile([128, 4, 65], FP32, tag='psE',
                                      name=f'pa_{i}_{g}')
                        pas.append(pa)
                        for hh in range(4):
                            h = 4 * g + hh
                            nc.tensor.matmul(
                                pa[:, hh, :], lhsT=qq8[:, h, rsl],
                                rhs=kvb_bf[:, h * 65:(h + 1) * 65],
                                start=True, stop=True)
                        nc.vector.reciprocal(out=z8[:, gg * 4:(gg + 1) * 4],
                                             in_=pa[:, :, 64])
                    for gg in range(2):
                        g = 2 * p + gg
                        zb = z8[:, gg * 4:(gg + 1) * 4].broadcast_to(
                            (128, 4, 64))
                        nc.vector.tensor_tensor(
                            out=yt_all[:, i, g * 4:(g + 1) * 4, :],
                            in0=pas[gg][:, :, 0:64], in1=zb,
                            op=AluOpType.mult)
                    hsl = slice(p * 512, (p + 1) * 512)
                    ytr = yt_all[:, i, :, :].rearrange('p h d -> p (h d)')
                    nc.vector.tensor_tensor(out=ytr[:, hsl], in0=ytr[:, hsl],
                                            in1=qn_sb[:, i, hsl],
                                            op=AluOpType.add)

                def emit_ln2(i):
                    rsl = slice(i * 128, (i + 1) * 128)
                    ytf = yt_all[:, i, :, :].rearrange('p h d -> p (h d)')
                    asum = efp.tile([128, 1], FP32, tag='easum')
                    sqs = efp.tile([128, 1], FP32, tag='esqs')
                    nc.scalar.activation(out=ascr, in_=ytf, func=AF.Copy,
                                         accum_out=asum)
                    nc.scalar.activation(out=ascr, in_=ytf, func=AF.Square,
                                         accum_out=sqs)
                    mu2 = efp.tile([128, 1], FP32, tag='emu')
                    nc.vector.tensor_scalar_mul(out=mu2, in0=asum,
                                                scalar1=INV_E)
                    m2 = efp.tile([128, 1], FP32, tag='em2')
                    nc.vector.tensor_tensor(out=m2, in0=mu2, in1=mu2,
                                            op=AluOpType.mult)
                    var = efp.tile([128, 1], FP32, tag='evar')
                    nc.vector.tensor_scalar(out=var, in0=sqs, scalar1=INV_E,
                                            scalar2=m2,
                                            op0=AluOpType.mult,
                                            op1=AluOpType.subtract)
                    sd = efp.tile([128, 1], FP32, tag='esd')
                    nc.scalar.activation(out=sd, in_=var, func=AF.Sqrt,
                                         bias=eps1, scale=1.0)
                    rstd2 = efp.tile([128, 1], FP32, tag='erstd')
                    nc.vector.reciprocal(out=rstd2, in_=sd)
                    nc.vector.tensor_scalar(out=xh_sb[:, i, :], in0=ytf,
                                            scalar1=mu2, scalar2=rstd2,
                                            op0=AluOpType.subtract,
                                            op1=AluOpType.mult)
                    nc.sync.dma_start(
                        out=xh_dram[rsl, :], in_=xh_sb[:, i, :])

                def emit_o(i):
                    for ch in range(2):
                        csl = slice(ch * 512, (ch + 1) * 512)
                        po = psg.tile([128, 512], FP32, tag='psG',
                                      name=f'po_{i}_{ch}')
                        for k2 in range(NK2):
                            nc.tensor.matmul(
                                po,
                                lhsT=xhT8[:, 2 * k2:2 * k2 + 2,
                                          i * 128:(i + 1) * 128],
                                rhs=wo8[:, 2 * k2:2 * k2 + 2, csl],
                                start=(k2 == 0), stop=(k2 == NK2 - 1),
                                perf_mode=DR)
                        ot2 = gop.tile([128, 512], BF, tag='ot2')
                        nc.scalar.activation(out=ot2, in_=po, func=AF.Copy,
                                             scale=SO)
                        ot = gop.tile([128, 512], FP32, tag='ot')
                        nc.vector.tensor_tensor(
                            out=ot, in0=ot2, in1=xh_sb[:, i, csl],
                            op=AluOpType.add)
                        nc.gpsimd.dma_start(
                            out=d_out[i * 128:(i + 1) * 128, csl], in_=ot)

                for i in range(NT):
                    emit_attn_pass(i, 0)
                for i in range(NT):
                    emit_attn_pass(i, 1)
                    emit_ln2(i)
                    if i == 3:
                        emit_xh_transpose(slice(0, 512), 0)
                for i in range(4):
                    emit_o(i)
                emit_xh_transpose(slice(512, 1024), 1)
                for i in range(4, NT):
                    emit_o(i)

    nc.compile()
    return nc


def _get_program():
    if 'p' not in _BUILD_CACHE:
        _BUILD_CACHE['p'] = _build_program()
    return _BUILD_CACHE['p']


def _phm_weight(A, S):
    f = A.shape[0]
    din, dout = f * S.shape[1], f * S.shape[2]
    W = np.einsum('nij,nkl->ikjl', np.asarray(A, np.float32),
                  np.asarray(S, np.float32))
    return np.ascontiguousarray(W.reshape(din, dout))


def _reference_np(query, qA, qS, qb, kA, kS, kb, vA, vS, vb, oA, oS, ob,
                  g1, b1, g2, b2):
    """Pure-numpy fallback (only used for non-default affine/bias inputs)."""
    x = np.asarray(query, np.float64)
    Lq, Nq, Eq = x.shape

    def ln(t, g, b, eps=1e-5):
        mu = t.mean(-1, keepdims=True)
        var = ((t - mu) ** 2).mean(-1, keepdims=True)
        return (t - mu) / np.sqrt(var + eps) * g + b

    def phm(t, A, S, bb):
        W = np.einsum('nij,nkl->ikjl', np.asarray(A, np.float64),
                      np.asarray(S, np.float64)).reshape(Eq, Eq)
        return t @ W + np.asarray(bb, np.float64)

    qn = ln(x, g1, b1)
    q = np.maximum(phm(qn, qA, qS, qb), 0)
    k = np.maximum(phm(x, kA, kS, kb), 0)
    v = phm(x, vA, vS, vb)
    resh = lambda t: t.reshape(Lq, Nq * H, D).transpose(1, 0, 2)
    q, k, v = resh(q), resh(k), resh(v)
    idx = (np.pi / 2) * np.arange(1, Lq + 1)[None, :, None] / Lq
    s, c = np.sin(idx), np.cos(idx)
    q_ = np.concatenate([q * s, q * c], -1)
    k_ = np.concatenate([k * s, k * c], -1)
    kv = np.einsum('nld,nlm->ndm', k_, v)
    z = 1.0 / np.maximum(np.einsum('nld,nd->nl', q_, k_.sum(1)), 1e-6)
    attn = np.einsum('nld,ndm->nlm', q_, kv) * z[..., None]
    attn = attn.transpose(1, 0, 2).reshape(Lq, Nq, Eq) + qn
    out = ln(attn, g2, b2)
    return (phm(out, oA, oS, ob) + out).astype(np.float32)


def kernel(**inputs):
    query = np.asarray(inputs['query'], np.float32)
    g1 = np.asarray(inputs['g1'], np.float32)
    b1 = np.asarray(inputs['b1'], np.float32)
    g2 = np.asarray(inputs['g2'], np.float32)
    b2 = np.asarray(inputs['b2'], np.float32)
    biases = [np.asarray(inputs[k], np.float32) for k in
              ('qb', 'kb', 'vb', 'ob')]

    default_affine = (np.all(g1 == 1.0) and np.all(b1 == 0.0)
                      and np.all(g2 == 1.0) and np.all(b2 == 0.0)
                      and all(not np.any(b) for b in biases))
    if not default_affine:
        return _reference_np(**inputs)

    Wq = _phm_weight(inputs['qA'], inputs['qS'])
    Wk = _phm_weight(inputs['kA'], inputs['kS'])
    Wv = _phm_weight(inputs['vA'], inputs['vS'])
    Wo = _phm_weight(inputs['oA'], inputs['oS'])
    sc = float(2.0 ** EW)
    for W in (Wq, Wk, Wv, Wo):
        assert np.abs(W).max() * sc < 240.0, "fp8 weight scale overflow"
    wq8 = (Wq * sc).astype(F8)
    wk8 = (Wk * sc).astype(F8)
    wv8 = (Wv * sc).astype(F8)
    wo8 = (Wo * sc).astype(F8)
    # -colsum(Wq) * 2^EW as a single row (partition 0)
    ncsw = (-Wq.sum(axis=0) * sc).astype(BF16).reshape(1, E)

    nc = _get_program()

    s_full = np.sin((np.pi / 2) * np.arange(1, L + 1, dtype=np.float32) / L)
    c_full = np.cos((np.pi / 2) * np.arange(1, L + 1, dtype=np.float32) / L)

    in_maps = []
    for core in range(NCORES):
        b = core // 2
        l0 = (core % 2) * T
        x = np.ascontiguousarray(query[l0:l0 + T, b, :])
        s = s_full[l0:l0 + T]
        c = c_full[l0:l0 + T]
        im = {
            'x_tm': x.astype(BF16),
            'x_fm': np.ascontiguousarray(x.T).astype(F8),
            'wq': wq8, 'wk': wk8, 'wv': wv8, 'wo': wo8,
            's_bcast': np.ascontiguousarray(
                np.broadcast_to(s * SQH, (128, T))).astype(BF16),
            'c_bcast': np.ascontiguousarray(
                np.broadcast_to(c * SQH, (128, T))).astype(BF16),
            's_cols': np.ascontiguousarray(s.reshape(NT, 128).T) * SKV,
            'c_cols': np.ascontiguousarray(c.reshape(NT, 128).T) * SKV,
            'neg_csw': ncsw,
        }
        in_maps.append(im)

    trace = bool(os.environ.get('KERNEL_TRACE'))
    kw = {}
    tdir = os.environ.get('KERNEL_TRACE_DIR')
    if trace and tdir:
        os.makedirs(tdir, exist_ok=True)
        kw['tmpdir'] = tdir
    res = run_bass_kernel_spmd(nc, in_maps, list(range(NCORES)), trace=trace,
                               **kw)
    kernel._last_exec_ns = res.exec_time_ns
    kernel._last_res = res

    out = np.empty((L, N, E), np.float32)
    for core in range(NCORES):
        b = core // 2
        l0 = (core % 2) * T
        out[l0:l0 + T, b, :] = res.results[core]['out']
    return out


kernel._last_exec_ns = None


# revision 65
# speedup vs baseline: 1.0152x; 1.0152x over previous
"""Cosformer attention Bass kernel for 8 trn2 NeuronCores.

Sharding: core c handles batch c//2, sequence half c%2 (1024 tokens).
Per-head linear-attention state (kv, ksum) is AllReduce'd (bf16) between
the two cores sharing a batch.

v3: feature-major q projection fed directly from x_fm8 with LN1 folded
in as a rank-1 (mu x colsum(Wq)) correction + rstd folded into the s/c
position tensors -- this removes the qn DRAM round-trip/transposes and
makes the q GEMM independent of LN1, so it hides the kv AllReduce.
Wo+I is folded host-side (removes the final residual pass). LN stats
run on ACT (Square+accum_out); epilogues are split across DVE/ACT/
GPSIMD so the PE never starves and HAM stays warm.
"""

import sys

for _p in ('/opt/trn_rl_repo',):
    if _p not in sys.path:
        sys.path.insert(0, _p)

import importlib.util as _ilu
import os

os.environ.setdefault('NEURON_RT_RESET_CORES', '1')


# The image's antenv may lack axon_hooks (needed for trace=True); register ours.
def _ensure_axon_hooks():
    try:
        import antenv.axon_hooks  # noqa: F401
        return
    except ImportError:
        pass
    for _hp in ('/opt/trn_rl_repo/antenv/axon_hooks.py',):
        if os.path.exists(_hp):
            _spec = _ilu.spec_from_file_location('antenv.axon_hooks', _hp)
            _mod = _ilu.module_from_spec(_spec)
            _spec.loader.exec_module(_mod)
            sys.modules['antenv.axon_hooks'] = _mod
            return
    import types
    _mod = types.ModuleType('antenv.axon_hooks')
    _mod._hook = None
    def _set(h):
        _mod._hook = h
    def _get():
        return _mod._hook
    _mod.set_axon_ntff_profile_hook = _set
    _mod.get_axon_ntff_profile_hook = _get
    sys.modules['antenv.axon_hooks'] = _mod
    try:
        from trn_agent_boot.trn_boot import _ntff_profile_via_ctypes
        _h = _ntff_profile_via_ctypes('/opt/axon/libaxon_pjrt.so')
        if _h is not None:
            _mod._hook = _h
    except Exception:
        pass


_ensure_axon_hooks()

import numpy as np
import ml_dtypes

import concourse.bass as bass
import concourse.tile as tile
from concourse import bacc, mybir
from concourse.alu_op_type import AluOpType
from concourse.bass_utils import run_bass_kernel_spmd

BF16 = ml_dtypes.bfloat16
F8 = ml_dtypes.float8_e4m3
FP32 = mybir.dt.float32
BF = mybir.dt.bfloat16
F8E4 = mybir.dt.float8e4
AF = mybir.ActivationFunctionType
DR = mybir.MatmulPerfMode.DoubleRow

L, N, E, H, D = 2048, 4, 1024, 16, 64
T = 1024            # tokens per core
NT = T // 128       # 8 token tiles
NK = E // 128       # 8 contraction tiles
NK2 = NK // 2       # 4 DoubleRow pair steps
NJ = E // 128       # 8 output-feature tiles
NCORES = 8
EPS_LN = 1e-5

EW = 8              # weight scale exponent: W8 = W * 2^EW
SKV = 2.0 ** (2 - EW)   # k/v epilogue scale (psum -> activation)
SO = 2.0 ** (-EW)       # o epilogue scale
SQH = 2.0 ** (-3)       # host fold into s/c tensors: q ends up x 2^5

_BUILD_CACHE = {}


def _build_program():
    nc = bacc.Bacc("TRN2", target_bir_lowering=False, debug=False,
                   num_devices=NCORES)

    # ---- DRAM I/O ----
    d_x_tm = nc.dram_tensor('x_tm', [T, E], BF, kind='ExternalInput')
    d_x_fm = nc.dram_tensor('x_fm', [E, T], F8E4, kind='ExternalInput')
    d_wq = nc.dram_tensor('wq', [E, E], F8E4, kind='ExternalInput')
    d_wk = nc.dram_tensor('wk', [E, E], F8E4, kind='ExternalInput')
    d_wv = nc.dram_tensor('wv', [E, E], F8E4, kind='ExternalInput')
    d_wo = nc.dram_tensor('wo', [E, E], F8E4, kind='ExternalInput')
    d_sb = nc.dram_tensor('s_bcast', [128, T], BF, kind='ExternalInput')
    d_cb = nc.dram_tensor('c_bcast', [128, T], BF, kind='ExternalInput')
    d_scol = nc.dram_tensor('s_cols', [128, NT], FP32, kind='ExternalInput')
    d_ccol = nc.dram_tensor('c_cols', [128, NT], FP32, kind='ExternalInput')
    d_csw = nc.dram_tensor('neg_csw', [1, E], BF, kind='ExternalInput')
    d_out = nc.dram_tensor('out', [T, E], FP32, kind='ExternalOutput')

    RG = [[0, 1], [2, 3], [4, 5], [6, 7]]
    INV_E = 1.0 / float(E)

    with tile.TileContext(nc) as tc:
        with (
            tc.tile_pool(name='persist', bufs=1) as pp,
            tc.tile_pool(name='dram', bufs=1, space='DRAM') as dp,
        ):
            # ---- persistent tiles ----
            sbt = pp.tile([128, T], BF, tag='sbt')
            cbt = pp.tile([128, T], BF, tag='cbt')
            scol = pp.tile([128, NT], FP32, tag='scol')
            ccol = pp.tile([128, NT], FP32, tag='ccol')
            cswt = pp.tile([1, E], BF, tag='cswt')
            eps1 = pp.tile([128, 1], FP32, tag='eps1')
            wk8 = pp.tile([128, NK, E], F8E4, tag='wk8')
            wv8 = pp.tile([128, NK, E], F8E4, tag='wv8')
            wq8 = pp.tile([128, NK, E], F8E4, tag='wq8')
            wo8 = pp.tile([128, NK, E], F8E4, tag='wo8')
            xfm8 = pp.tile([128, NK, T], F8E4, tag='xfm8')
            ksc8 = pp.tile([128, NT, H, 128], F8E4, tag='ksc8')
            vaug8 = pp.tile([128, NT, H, 65], F8E4, tag='vaug8')
            qn_sb = pp.tile([128, NT, E], BF, tag='qn_sb')
            qq8 = pp.tile([128, H, T], BF, tag='qq8')
            kvp = pp.tile([128, H * 65], BF, tag='kvp')
            yt_all = pp.tile([128, NT, H, 64], BF, tag='yt_all')
            xh_sb = pp.tile([128, NT, E], BF, tag='xh_sb')
            xhT8 = pp.tile([128, NK, T], F8E4, tag='xhT8')
            wz = pp.tile([128, 512], BF, tag='wz')
            ascr = pp.tile([128, E], BF, tag='ascr')       # ACT accum scratch

            # DRAM scratch
            xh_dram = dp.tile([T, E], BF)
            mrs_dram = dp.tile([T, 128], BF)
            kv_cc_in1 = dp.tile([128, 8 * 65], BF)
            kv_cc_out1 = dp.tile([128, 8 * 65], BF)
            kv_cc_in2 = dp.tile([128, 8 * 65], BF)
            kv_cc_out2 = dp.tile([128, 8 * 65], BF)

            # ---- PE warm-up: dense matmuls on a zeroed tile ----
            nc.vector.memset(wz, 0.0)
            nc.vector.memset(eps1, EPS_LN)
            # ones column carries the NET v scale (2^EW * SKV = 2^2)
            nc.gpsimd.memset(vaug8[:, :, :, 64:65], 2.0 ** 2)
            with tc.tile_pool(name='psW', bufs=1, space='PSUM') as psw:
                pw = psw.tile([128, 512], FP32, tag='psW')
                for _ in range(28):
                    nc.tensor.matmul(pw, lhsT=wz[:, 0:128], rhs=wz,
                                     start=True, stop=True)

            # ---- input DMAs (order per queue == priority) ----
            xfm_src = d_x_fm[:].rearrange('(k p) t -> p k t', p=128)
            wk_src = d_wk[:].rearrange('(k p) e -> p k e', p=128)
            wv_src = d_wv[:].rearrange('(k p) e -> p k e', p=128)
            wq_src = d_wq[:].rearrange('(k p) e -> p k e', p=128)
            wo_src = d_wo[:].rearrange('(k p) e -> p k e', p=128)
            for k in range(NK):
                nc.sync.dma_start(out=xfm8[:, k, :], in_=xfm_src[:, k, :])
                nc.scalar.dma_start(out=wk8[:, k, :], in_=wk_src[:, k, :])
                nc.gpsimd.dma_start(out=wv8[:, k, :], in_=wv_src[:, k, :])
            nc.sync.dma_start(out=scol, in_=d_scol[:])
            nc.sync.dma_start(out=ccol, in_=d_ccol[:])
            nc.sync.dma_start(out=sbt, in_=d_sb[:])
            nc.sync.dma_start(out=cbt, in_=d_cb[:])
            nc.sync.dma_start(out=cswt, in_=d_csw[:])
            for k in range(NK):
                nc.gpsimd.dma_start(out=wq8[:, k, :], in_=wq_src[:, k, :])
            for k in range(NK):
                nc.gpsimd.dma_start(out=wo8[:, k, :], in_=wo_src[:, k, :])

            # ================= A phase: LN1 stats + qn (token-major) ========
            # stats via ACT Square/Copy + accum_out; qn via GPSIMD
            with tc.tile_pool(name='ln1', bufs=3) as ap:
                for i in range(NT):
                    xt = ap.tile([128, E], BF, tag='xt')
                    nc.sync.dma_start(out=xt,
                                      in_=d_x_tm[i * 128:(i + 1) * 128, :])
                    st = ap.tile([128, 2, 6], FP32, tag='st')
                    xg = xt[:].rearrange('p (g d) -> p g d', g=2)
                    nc.vector.bn_stats(out=st[:, 0, :], in_=xg[:, 0, :])
                    nc.vector.bn_stats(out=st[:, 1, :], in_=xg[:, 1, :])
                    mv = ap.tile([128, 2], FP32, tag='mv')
                    nc.vector.bn_aggr(out=mv, in_=st)
                    sd = ap.tile([128, 1], FP32, tag='sd')
                    nc.scalar.activation(out=sd, in_=mv[:, 1:2], func=AF.Sqrt,
                                         bias=eps1, scale=1.0)
                    rstd = ap.tile([128, 1], FP32, tag='rstd')
                    nc.vector.reciprocal(out=rstd, in_=sd)
                    mub = ap.tile([128, 1], BF, tag='mub')
                    nc.vector.tensor_copy(out=mub, in_=mv[:, 0:1])
                    nc.sync.dma_start(
                        out=mrs_dram[i * 128:(i + 1) * 128, 0:1], in_=mub)
                    nc.vector.tensor_scalar(out=qn_sb[:, i, :], in0=xt,
                                            scalar1=mv[:, 0:1], scalar2=rstd,
                                            op0=AluOpType.subtract,
                                            op1=AluOpType.mult)
            # mu row (partition 0) for the rank-1 LN1 fold in the q GEMM.
            # rstd needs no handling at all: a per-token scale on q cancels
            # in the attention num/den ratio, and relu commutes with it.
            bcT = pp.tile([128, T], BF, tag='bcT')
            nc.sync.dma_start(out=bcT, in_=mrs_dram[:], transpose=True)

            # ================= B phase: fused k/v proj + kv accumulation ====
            with (
                tc.tile_pool(name='psB', bufs=2, space='PSUM') as psb,
                tc.tile_pool(name='psKV', bufs=4, space='PSUM') as pskv,
            ):
                kvt = [pskv.tile([128, 4, 65], FP32, tag='psKV',
                                 name=f'kvt_{g}') for g in range(4)]

                for i in range(NT):
                    pk = psb.tile([128, 1024], FP32, tag='psB', name=f'pk_{i}')
                    pv = psb.tile([128, 1024], FP32, tag='psB', name=f'pv_{i}')
                    lhs = lambda k2: xfm8[:, 2 * k2:2 * k2 + 2,
                                          i * 128:(i + 1) * 128]
                    for k2 in range(NK2):
                        for ch in range(2):
                            csl = slice(ch * 512, (ch + 1) * 512)
                            nc.tensor.matmul(
                                pk[:, csl], lhsT=lhs(k2),
                                rhs=wk8[:, 2 * k2:2 * k2 + 2, csl],
                                start=(k2 == 0), stop=(k2 == NK2 - 1),
                                perf_mode=DR)
                    for k2 in range(NK2):
                        for ch in range(2):
                            csl = slice(ch * 512, (ch + 1) * 512)
                            nc.tensor.matmul(
                                pv[:, csl], lhsT=lhs(k2),
                                rhs=wv8[:, 2 * k2:2 * k2 + 2, csl],
                                start=(k2 == 0), stop=(k2 == NK2 - 1),
                                perf_mode=DR)
                    # k epilogue on ACT: relu(scol*pk) == scol*relu(pk)
                    # (scol, ccol >= 0)
                    for ch in range(2):
                        csl = slice(ch * 512, (ch + 1) * 512)
                        pkv = pk[:, csl].rearrange('p (h d) -> p h d', d=64)
                        nc.scalar.activation(
                            out=ksc8[:, i, ch * 8:(ch + 1) * 8, 0:64],
                            in_=pkv, func=AF.Relu, scale=scol[:, i:i + 1])
                        nc.scalar.activation(
                            out=ksc8[:, i, ch * 8:(ch + 1) * 8, 64:128],
                            in_=pkv, func=AF.Relu, scale=ccol[:, i:i + 1])
                        nc.scalar.activation(
                            out=vaug8[:, i, ch * 8:(ch + 1) * 8, 0:64],
                            in_=pv[:, csl].rearrange('p (h d) -> p h d', d=64),
                            func=AF.Copy, scale=SKV)
                    # kv accumulation interleaved per token tile. Only the
                    # very first MM per PSUM bank carries start=True (bank
                    # clear); i=0 writes then land on has_written=0 regions
                    # (overwrite), i>0 accumulate.
                    for g in range(4):
                        for hh in range(4):
                            h = 4 * g + hh
                            nc.tensor.matmul(
                                kvt[g][:, hh, :],
                                lhsT=ksc8[:, i, h, :],
                                rhs=vaug8[:, i, h, :],
                                start=(i == 0 and hh == 0),
                                stop=(i == NT - 1))
                # split AllReduce: heads 0-7 first. PSUM->SBUF copies on ACT
                # (must not queue behind DVE work).
                kvv = kvp[:].rearrange('p (h c) -> p h c', c=65)
                for g in range(2):
                    nc.scalar.activation(out=kvv[:, g * 4:(g + 1) * 4, :],
                                         in_=kvt[g], func=AF.Copy)
                nc.gpsimd.dma_start(out=kv_cc_in1[:], in_=kvp[:, 0:520])
                nc.gpsimd.collective_compute(
                    'AllReduce', AluOpType.add,
                    ins=[kv_cc_in1.opt()], outs=[kv_cc_out1.opt()],
                    replica_groups=RG)
                for g in range(2, 4):
                    nc.scalar.activation(out=kvv[:, g * 4:(g + 1) * 4, :],
                                         in_=kvt[g], func=AF.Copy)
                nc.gpsimd.dma_start(out=kv_cc_in2[:], in_=kvp[:, 520:1040])
                nc.gpsimd.collective_compute(
                    'AllReduce', AluOpType.add,
                    ins=[kv_cc_in2.opt()], outs=[kv_cc_out2.opt()],
                    replica_groups=RG)

            kvb_bf = pp.tile([128, H * 65], BF, tag='kvb_bf')
            nc.gpsimd.dma_start(out=kvb_bf[:, 0:520], in_=kv_cc_out1[:])
            nc.gpsimd.dma_start(out=kvb_bf[:, 520:1040], in_=kv_cc_out2[:])

            # ================= D phase: q projection (feature-major) ========
            # pq[j,t] = (x @ Wq)[j,t] * 2^8 ; LN1 folds in via
            # t1 = pq + mu_b * (-colsum(Wq) * 2^8), then relu, then
            # qq8 = relu(t1) * (s|c * rstd * 2^-3)
            with (
                tc.tile_pool(name='psD', bufs=3, space='PSUM') as psd,
                tc.tile_pool(name='qsb', bufs=3) as qsb,
            ):
                for j in range(NJ):
                    pq = psd.tile([128, 1024], FP32, tag='psD',
                                  name=f'pq_{j}')
                    for k2 in range(NK2):
                        for ch in range(2):
                            csl = slice(ch * 512, (ch + 1) * 512)
                            nc.tensor.matmul(
                                pq[:, csl],
                                lhsT=wq8[:, 2 * k2:2 * k2 + 2,
                                         j * 128:(j + 1) * 128],
                                rhs=xfm8[:, 2 * k2:2 * k2 + 2, csl],
                                start=(k2 == 0), stop=False,
                                perf_mode=DR)
                    # rank-1 LN1 fold: pq += (-colsum(Wq) row)^T @ mu row
                    for ch in range(2):
                        csl = slice(ch * 512, (ch + 1) * 512)
                        nc.tensor.matmul(
                            pq[:, csl],
                            lhsT=cswt[0:1, j * 128:(j + 1) * 128],
                            rhs=bcT[0:1, csl], start=False, stop=True)
                    qrel = qsb.tile([128, 1024], BF, tag='qrel')
                    nc.scalar.activation(out=qrel, in_=pq, func=AF.Relu)
                    h0, h1 = 2 * j, 2 * j + 1
                    nc.vector.tensor_tensor(
                        out=qq8[0:64, h0, :], in0=qrel[0:64, :],
                        in1=sbt[0:64, :], op=AluOpType.mult)
                    nc.vector.tensor_tensor(
                        out=qq8[64:128, h0, :], in0=qrel[0:64, :],
                        in1=cbt[0:64, :], op=AluOpType.mult)
                    nc.vector.tensor_tensor(
                        out=qq8[0:64, h1, :], in0=qrel[64:128, :],
                        in1=sbt[64:128, :], op=AluOpType.mult)
                    nc.vector.tensor_tensor(
                        out=qq8[64:128, h1, :], in0=qrel[64:128, :],
                        in1=cbt[64:128, :], op=AluOpType.mult)

            # ================= E phase: attention + LN2 (token-major) =======
            xhT = pp.tile([128, NJ, 512], BF, tag='xhT')

            def emit_xh_transpose(tsl, par):
                for j in range(NJ):
                    eng = nc.sync if j % 2 == par else nc.scalar
                    eng.dma_start(out=xhT[:, j, :],
                                  in_=xh_dram[tsl, j * 128:(j + 1) * 128],
                                  transpose=True)
                    nc.scalar.activation(out=xhT8[:, j, tsl],
                                         in_=xhT[:, j, :], func=AF.Copy)

            with (
                tc.tile_pool(name='psE', bufs=6, space='PSUM') as pse,
                tc.tile_pool(name='ef', bufs=3) as efp,
                tc.tile_pool(name='psG', bufs=2, space='PSUM') as psg,
                tc.tile_pool(name='go', bufs=3) as gop,
            ):
                def emit_attn_pass(i, p):
                    rsl = slice(i * 128, (i + 1) * 128)
                    z8 = efp.tile([128, 8], FP32, tag='z8',
                                  name=f'z8_{i}_{p}')
                    pas = []
                    for gg in range(2):
                        g = 2 * p + gg
                        pa = pse.tile([128, 4, 65], FP32, tag='psE',
                                      name=f'pa_{i}_{g}')
                        pas.append(pa)
                        for hh in range(4):
                            h = 4 * g + hh
                            nc.tensor.matmul(
                                pa[:, hh, :], lhsT=qq8[:, h, rsl],
                                rhs=kvb_bf[:, h * 65:(h + 1) * 65],
                                start=True, stop=True)
                        nc.vector.reciprocal(out=z8[:, gg * 4:(gg + 1) * 4],
                                             in_=pa[:, :, 64])
                    for gg in range(2):
                        g = 2 * p + gg
                        zb = z8[:, gg * 4:(gg + 1) * 4].broadcast_to(
                            (128, 4, 64))
                        nc.vector.tensor_tensor(
                            out=yt_all[:, i, g * 4:(g + 1) * 4, :],
                            in0=pas[gg][:, :, 0:64], in1=zb,
                            op=AluOpType.mult)
                    hsl = slice(p * 512, (p + 1) * 512)
                    ytr = yt_all[:, i, :, :].rearrange('p h d -> p (h d)')
                    nc.vector.tensor_tensor(out=ytr[:, hsl], in0=ytr[:, hsl],
                                            in1=qn_sb[:, i, hsl],
                                            op=AluOpType.add)

                def emit_ln2(i):
                    rsl = slice(i * 128, (i + 1) * 128)
                    ytf = yt_all[:, i, :, :].rearrange('p h d -> p (h d)')
                    asum = efp.tile([128, 1], FP32, tag='easum')
                    sqs = efp.tile([128, 1], FP32, tag='esqs')
                    nc.scalar.activation(out=ascr, in_=ytf, func=AF.Copy,
                                         accum_out=asum)
                    nc.scalar.activation(out=ascr, in_=ytf, func=AF.Square,
                                         accum_out=sqs)
                    mu2 = efp.tile([128, 1], FP32, tag='emu')
                    nc.vector.tensor_scalar_mul(out=mu2, in0=asum,
                                                scalar1=INV_E)
                    m2 = efp.tile([128, 1], FP32, tag='em2')
                    nc.vector.tensor_tensor(out=m2, in0=mu2, in1=mu2,
                                            op=AluOpType.mult)
                    var = efp.tile([128, 1], FP32, tag='evar')
                    nc.vector.tensor_scalar(out=var, in0=sqs, scalar1=INV_E,
                                            scalar2=m2,
                                            op0=AluOpType.mult,
                                            op1=AluOpType.subtract)
                    sd = efp.tile([128, 1], FP32, tag='esd')
                    nc.scalar.activation(out=sd, in_=var, func=AF.Sqrt,
                                         bias=eps1, scale=1.0)
                    rstd2 = efp.tile([128, 1], FP32, tag='erstd')
                    nc.vector.reciprocal(out=rstd2, in_=sd)
                    nc.vector.tensor_scalar(out=xh_sb[:, i, :], in0=ytf,
                                            scalar1=mu2, scalar2=rstd2,
                                            op0=AluOpType.subtract,
                                            op1=AluOpType.mult)
                    nc.gpsimd.dma_start(
                        out=xh_dram[rsl, :], in_=xh_sb[:, i, :])

                def emit_o(i):
                    for ch in range(2):
                        csl = slice(ch * 512, (ch + 1) * 512)
                        po = psg.tile([128, 512], FP32, tag='psG',
                                      name=f'po_{i}_{ch}')
                        for k2 in range(NK2):
                            nc.tensor.matmul(
                                po,
                                lhsT=xhT8[:, 2 * k2:2 * k2 + 2,
                                          i * 128:(i + 1) * 128],
                                rhs=wo8[:, 2 * k2:2 * k2 + 2, csl],
                                start=(k2 == 0), stop=(k2 == NK2 - 1),
                                perf_mode=DR)
                        ot2 = gop.tile([128, 512], BF, tag='ot2')
                        nc.scalar.activation(out=ot2, in_=po, func=AF.Copy,
                                             scale=SO)
                        ot = gop.tile([128, 512], FP32, tag='ot')
                        nc.vector.tensor_tensor(
                            out=ot, in0=ot2, in1=xh_sb[:, i, csl],
                            op=AluOpType.add)
                        nc.sync.dma_start(
                            out=d_out[i * 128:(i + 1) * 128, csl], in_=ot)

                for i in range(NT):
                    emit_attn_pass(i, 0)
                for i in range(NT):
                    emit_attn_pass(i, 1)
                    emit_ln2(i)
                    if i == 3:
                        emit_xh_transpose(slice(0, 512), 0)
                for i in range(4):
                    emit_o(i)
                emit_xh_transpose(slice(512, 1024), 1)
                for i in range(4, NT):
                    emit_o(i)

    nc.compile()
    return nc


def _get_program():
    if 'p' not in _BUILD_CACHE:
        _BUILD_CACHE['p'] = _build_program()
    return _BUILD_CACHE['p']


def _phm_weight(A, S):
    f = A.shape[0]
    din, dout = f * S.shape[1], f * S.shape[2]
    W = np.einsum('nij,nkl->ikjl', np.asarray(A, np.float32),
                  np.asarray(S, np.float32))
    return np.ascontiguousarray(W.reshape(din, dout))


def _reference_np(query, qA, qS, qb, kA, kS, kb, vA, vS, vb, oA, oS, ob,
                  g1, b1, g2, b2):
    """Pure-numpy fallback (only used for non-default affine/bias inputs)."""
    x = np.asarray(query, np.float64)
    Lq, Nq, Eq = x.shape

    def ln(t, g, b, eps=1e-5):
        mu = t.mean(-1, keepdims=True)
        var = ((t - mu) ** 2).mean(-1, keepdims=True)
        return (t - mu) / np.sqrt(var + eps) * g + b

    def phm(t, A, S, bb):
        W = np.einsum('nij,nkl->ikjl', np.asarray(A, np.float64),
                      np.asarray(S, np.float64)).reshape(Eq, Eq)
        return t @ W + np.asarray(bb, np.float64)

    qn = ln(x, g1, b1)
    q = np.maximum(phm(qn, qA, qS, qb), 0)
    k = np.maximum(phm(x, kA, kS, kb), 0)
    v = phm(x, vA, vS, vb)
    resh = lambda t: t.reshape(Lq, Nq * H, D).transpose(1, 0, 2)
    q, k, v = resh(q), resh(k), resh(v)
    idx = (np.pi / 2) * np.arange(1, Lq + 1)[None, :, None] / Lq
    s, c = np.sin(idx), np.cos(idx)
    q_ = np.concatenate([q * s, q * c], -1)
    k_ = np.concatenate([k * s, k * c], -1)
    kv = np.einsum('nld,nlm->ndm', k_, v)
    z = 1.0 / np.maximum(np.einsum('nld,nd->nl', q_, k_.sum(1)), 1e-6)
    attn = np.einsum('nld,ndm->nlm', q_, kv) * z[..., None]
    attn = attn.transpose(1, 0, 2).reshape(Lq, Nq, Eq) + qn
    out = ln(attn, g2, b2)
    return (phm(out, oA, oS, ob) + out).astype(np.float32)


def kernel(**inputs):
    query = np.asarray(inputs['query'], np.float32)
    g1 = np.asarray(inputs['g1'], np.float32)
    b1 = np.asarray(inputs['b1'], np.float32)
    g2 = np.asarray(inputs['g2'], np.float32)
    b2 = np.asarray(inputs['b2'], np.float32)
    biases = [np.asarray(inputs[k], np.float32) for k in
              ('qb', 'kb', 'vb', 'ob')]

    default_affine = (np.all(g1 == 1.0) and np.all(b1 == 0.0)
                      and np.all(g2 == 1.0) and np.all(b2 == 0.0)
                      and all(not np.any(b) for b in biases))
    if not default_affine:
        return _reference_np(**inputs)

    Wq = _phm_weight(inputs['qA'], inputs['qS'])
    Wk = _phm_weight(inputs['kA'], inputs['kS'])
    Wv = _phm_weight(inputs['vA'], inputs['vS'])
    Wo = _phm_weight(inputs['oA'], inputs['oS'])
    sc = float(2.0 ** EW)
    for W in (Wq, Wk, Wv, Wo):
        assert np.abs(W).max() * sc < 240.0, "fp8 weight scale overflow"
    wq8 = (Wq * sc).astype(F8)
    wk8 = (Wk * sc).astype(F8)
    wv8 = (Wv * sc).astype(F8)
    wo8 = (Wo * sc).astype(F8)
    # -colsum(Wq) * 2^EW as a single row (partition 0)
    ncsw = (-Wq.sum(axis=0) * sc).astype(BF16).reshape(1, E)

    nc = _get_program()

    s_full = np.sin((np.pi / 2) * np.arange(1, L + 1, dtype=np.float32) / L)
    c_full = np.cos((np.pi / 2) * np.arange(1, L + 1, dtype=np.float32) / L)

    in_maps = []
    for core in range(NCORES):
        b = core // 2
        l0 = (core % 2) * T
        x = np.ascontiguousarray(query[l0:l0 + T, b, :])
        s = s_full[l0:l0 + T]
        c = c_full[l0:l0 + T]
        im = {
            'x_tm': x.astype(BF16),
            'x_fm': np.ascontiguousarray(x.T).astype(F8),
            'wq': wq8, 'wk': wk8, 'wv': wv8, 'wo': wo8,
            's_bcast': np.ascontiguousarray(
                np.broadcast_to(s * SQH, (128, T))).astype(BF16),
            'c_bcast': np.ascontiguousarray(
                np.broadcast_to(c * SQH, (128, T))).astype(BF16),
            's_cols': np.ascontiguousarray(s.reshape(NT, 128).T) * SKV,
            'c_cols': np.ascontiguousarray(c.reshape(NT, 128).T) * SKV,
            'neg_csw': ncsw,
        }
        in_maps.append(im)

    trace = bool(os.environ.get('KERNEL_TRACE'))
    kw = {}
    tdir = os.environ.get('KERNEL_TRACE_DIR')
    if trace and tdir:
        os.makedirs(tdir, exist_ok=True)
        kw['tmpdir'] = tdir
    res = run_bass_kernel_spmd(nc, in_maps, list(range(NCORES)), trace=trace,
                               **kw)
    kernel._last_exec_ns = res.exec_time_ns
    kernel._last_res = res

    out = np.empty((L, N, E), np.float32)
    for core in range(NCORES):
        b = core // 2
        l0 = (core % 2) * T
        out[l0:l0 + T, b, :] = res.results[core]['out']
    return out


kernel._last_exec_ns = None


# revision 68
# speedup vs baseline: 1.1165x; 1.0998x over previous
"""Cosformer attention Bass kernel for 8 trn2 NeuronCores.

Sharding: core c handles batch c//2, sequence half c%2 (1024 tokens).
Per-head linear-attention state (kv, ksum) is AllReduce'd (bf16) between
the two cores sharing a batch.

v3: feature-major q projection fed directly from x_fm8 with LN1 folded
in as a rank-1 (mu x colsum(Wq)) correction + rstd folded into the s/c
position tensors -- this removes the qn DRAM round-trip/transposes and
makes the q GEMM independent of LN1, so it hides the kv AllReduce.
Wo+I is folded host-side (removes the final residual pass). LN stats
run on ACT (Square+accum_out); epilogues are split across DVE/ACT/
GPSIMD so the PE never starves and HAM stays warm.
"""

import sys

for _p in ('/opt/trn_rl_repo',):
    if _p not in sys.path:
        sys.path.insert(0, _p)

import importlib.util as _ilu
import os

os.environ.setdefault('NEURON_RT_RESET_CORES', '1')


# The image's antenv may lack axon_hooks (needed for trace=True); register ours.
def _ensure_axon_hooks():
    try:
        import antenv.axon_hooks  # noqa: F401
        return
    except ImportError:
        pass
    for _hp in ('/opt/trn_rl_repo/antenv/axon_hooks.py',):
        if os.path.exists(_hp):
            _spec = _ilu.spec_from_file_location('antenv.axon_hooks', _hp)
            _mod = _ilu.module_from_spec(_spec)
            _spec.loader.exec_module(_mod)
            sys.modules['antenv.axon_hooks'] = _mod
            return
    import types
    _mod = types.ModuleType('antenv.axon_hooks')
    _mod._hook = None
    def _set(h):
        _mod._hook = h
    def _get():
        return _mod._hook
    _mod.set_axon_ntff_profile_hook = _set
    _mod.get_axon_ntff_profile_hook = _get
    sys.modules['antenv.axon_hooks'] = _mod
    try:
        from trn_agent_boot.trn_boot import _ntff_profile_via_ctypes
        _h = _ntff_profile_via_ctypes('/opt/axon/libaxon_pjrt.so')
        if _h is not None:
            _mod._hook = _h
    except Exception:
        pass


_ensure_axon_hooks()

import numpy as np
import ml_dtypes

import concourse.bass as bass
import concourse.tile as tile
from concourse import bacc, mybir
from concourse.alu_op_type import AluOpType
from concourse.bass_utils import run_bass_kernel_spmd

BF16 = ml_dtypes.bfloat16
F8 = ml_dtypes.float8_e4m3
FP32 = mybir.dt.float32
BF = mybir.dt.bfloat16
F8E4 = mybir.dt.float8e4
AF = mybir.ActivationFunctionType
DR = mybir.MatmulPerfMode.DoubleRow

L, N, E, H, D = 2048, 4, 1024, 16, 64
T = 1024            # tokens per core
NT = T // 128       # 8 token tiles
NK = E // 128       # 8 contraction tiles
NK2 = NK // 2       # 4 DoubleRow pair steps
NJ = E // 128       # 8 output-feature tiles
NCORES = 8
EPS_LN = 1e-5

EW = 8              # weight scale exponent: W8 = W * 2^EW
SKV = 2.0 ** (2 - EW)   # k/v epilogue scale (psum -> activation)
SO = 2.0 ** (-EW)       # o epilogue scale
SQH = 2.0 ** (-3)       # host fold into s/c tensors: q ends up x 2^5

_BUILD_CACHE = {}


def _build_program():
    nc = bacc.Bacc("TRN2", target_bir_lowering=False, debug=False,
                   num_devices=NCORES)

    # ---- DRAM I/O ----
    d_x_tm = nc.dram_tensor('x_tm', [T, E], BF, kind='ExternalInput')
    d_x_fm = nc.dram_tensor('x_fm', [E, T], F8E4, kind='ExternalInput')
    d_wq = nc.dram_tensor('wq', [E, E], F8E4, kind='ExternalInput')
    d_wk = nc.dram_tensor('wk', [E, E], F8E4, kind='ExternalInput')
    d_wv = nc.dram_tensor('wv', [E, E], F8E4, kind='ExternalInput')
    d_wo = nc.dram_tensor('wo', [E, E], F8E4, kind='ExternalInput')
    d_sb = nc.dram_tensor('s_bcast', [128, T], BF, kind='ExternalInput')
    d_cb = nc.dram_tensor('c_bcast', [128, T], BF, kind='ExternalInput')
    d_scol = nc.dram_tensor('s_cols', [128, NT], FP32, kind='ExternalInput')
    d_ccol = nc.dram_tensor('c_cols', [128, NT], FP32, kind='ExternalInput')
    d_csw = nc.dram_tensor('neg_csw', [1, E], BF, kind='ExternalInput')
    d_out = nc.dram_tensor('out', [T, E], FP32, kind='ExternalOutput')

    RG = [[0, 1], [2, 3], [4, 5], [6, 7]]
    INV_E = 1.0 / float(E)

    with tile.TileContext(nc) as tc:
        with (
            tc.tile_pool(name='persist', bufs=1) as pp,
            tc.tile_pool(name='dram', bufs=1, space='DRAM') as dp,
        ):
            # ---- persistent tiles ----
            sbt = pp.tile([128, T], BF, tag='sbt')
            cbt = pp.tile([128, T], BF, tag='cbt')
            scol = pp.tile([128, NT], FP32, tag='scol')
            ccol = pp.tile([128, NT], FP32, tag='ccol')
            cswt = pp.tile([1, E], BF, tag='cswt')
            eps1 = pp.tile([128, 1], FP32, tag='eps1')
            wk8 = pp.tile([128, NK, E], F8E4, tag='wk8')
            wv8 = pp.tile([128, NK, E], F8E4, tag='wv8')
            wq8 = pp.tile([128, NK, E], F8E4, tag='wq8')
            wo8 = pp.tile([128, NK, E], F8E4, tag='wo8')
            xfm8 = pp.tile([128, NK, T], F8E4, tag='xfm8')
            ksc8 = pp.tile([128, NT, H, 128], F8E4, tag='ksc8')
            vaug8 = pp.tile([128, NT, H, 65], F8E4, tag='vaug8')
            qn_sb = pp.tile([128, NT, E], BF, tag='qn_sb')
            qq8 = pp.tile([128, H, T], BF, tag='qq8')
            kvp = pp.tile([128, H * 65], BF, tag='kvp')
            yt_all = pp.tile([128, NT, H, 64], BF, tag='yt_all')
            xh_sb = pp.tile([128, NT, E], BF, tag='xh_sb')
            xhT8 = pp.tile([128, NK, T], F8E4, tag='xhT8')
            wz = pp.tile([128, 512], BF, tag='wz')
            ascr = pp.tile([128, E], BF, tag='ascr')       # ACT accum scratch

            # DRAM scratch
            xh_dram = dp.tile([T, E], BF)
            mrs_dram = dp.tile([T, 128], BF)
            kv_cc_in1 = dp.tile([128, 8 * 65], BF)
            kv_cc_out1 = dp.tile([128, 8 * 65], BF)
            kv_cc_in2 = dp.tile([128, 8 * 65], BF)
            kv_cc_out2 = dp.tile([128, 8 * 65], BF)

            # ---- PE warm-up: dense matmuls on a zeroed tile ----
            nc.vector.memset(wz, 0.0)
            nc.vector.memset(eps1, EPS_LN)
            # ones column carries the NET v scale (2^EW * SKV = 2^2)
            nc.gpsimd.memset(vaug8[:, :, :, 64:65], 2.0 ** 2)
            with tc.tile_pool(name='psW', bufs=1, space='PSUM') as psw:
                pw = psw.tile([128, 512], FP32, tag='psW')
                for _ in range(28):
                    nc.tensor.matmul(pw, lhsT=wz[:, 0:128], rhs=wz,
                                     start=True, stop=True)

            # ---- input DMAs (order per queue == priority) ----
            xfm_src = d_x_fm[:].rearrange('(k p) t -> p k t', p=128)
            wk_src = d_wk[:].rearrange('(k p) e -> p k e', p=128)
            wv_src = d_wv[:].rearrange('(k p) e -> p k e', p=128)
            wq_src = d_wq[:].rearrange('(k p) e -> p k e', p=128)
            wo_src = d_wo[:].rearrange('(k p) e -> p k e', p=128)
            for k in range(NK):
                nc.sync.dma_start(out=xfm8[:, k, :], in_=xfm_src[:, k, :])
                nc.scalar.dma_start(out=wk8[:, k, :], in_=wk_src[:, k, :])
                nc.gpsimd.dma_start(out=wv8[:, k, :], in_=wv_src[:, k, :])
            nc.sync.dma_start(out=scol, in_=d_scol[:])
            nc.sync.dma_start(out=ccol, in_=d_ccol[:])
            nc.sync.dma_start(out=sbt, in_=d_sb[:])
            nc.sync.dma_start(out=cbt, in_=d_cb[:])
            nc.sync.dma_start(out=cswt, in_=d_csw[:])
            for k in range(NK):
                nc.gpsimd.dma_start(out=wq8[:, k, :], in_=wq_src[:, k, :])
            for k in range(NK):
                nc.gpsimd.dma_start(out=wo8[:, k, :], in_=wo_src[:, k, :])

            # ================= A phase: LN1 stats + qn (token-major) ========
            # stats via ACT Square/Copy + accum_out; qn via GPSIMD
            with tc.tile_pool(name='ln1', bufs=3) as ap:
                for i in range(NT):
                    xt = ap.tile([128, E], BF, tag='xt')
                    nc.sync.dma_start(out=xt,
                                      in_=d_x_tm[i * 128:(i + 1) * 128, :])
                    st = ap.tile([128, 2, 6], FP32, tag='st')
                    xg = xt[:].rearrange('p (g d) -> p g d', g=2)
                    nc.vector.bn_stats(out=st[:, 0, :], in_=xg[:, 0, :])
                    nc.vector.bn_stats(out=st[:, 1, :], in_=xg[:, 1, :])
                    mv = ap.tile([128, 2], FP32, tag='mv')
                    nc.vector.bn_aggr(out=mv, in_=st)
                    sd = ap.tile([128, 1], FP32, tag='sd')
                    nc.scalar.activation(out=sd, in_=mv[:, 1:2], func=AF.Sqrt,
                                         bias=eps1, scale=1.0)
                    rstd = ap.tile([128, 1], FP32, tag='rstd')
                    nc.vector.reciprocal(out=rstd, in_=sd)
                    mub = ap.tile([128, 1], BF, tag='mub')
                    nc.vector.tensor_copy(out=mub, in_=mv[:, 0:1])
                    nc.sync.dma_start(
                        out=mrs_dram[i * 128:(i + 1) * 128, 0:1], in_=mub)
                    nc.vector.tensor_scalar(out=qn_sb[:, i, :], in0=xt,
                                            scalar1=mv[:, 0:1], scalar2=rstd,
                                            op0=AluOpType.subtract,
                                            op1=AluOpType.mult)
            # mu row (partition 0) for the rank-1 LN1 fold in the q GEMM.
            # rstd needs no handling at all: a per-token scale on q cancels
            # in the attention num/den ratio, and relu commutes with it.
            bcT = pp.tile([128, T], BF, tag='bcT')
            nc.sync.dma_start(out=bcT, in_=mrs_dram[:], transpose=True)

            # ================= B phase: fused k/v proj + kv accumulation ====
            with (
                tc.tile_pool(name='psB', bufs=3, space='PSUM') as psb,
                tc.tile_pool(name='psKV', bufs=2, space='PSUM') as pskv,
            ):
                kvt = [pskv.tile([128, 4, 65], FP32, tag='psKV',
                                 name=f'kvt_{g}') for g in range(2)]

                for i in range(NT):
                    pk = psb.tile([128, 1024], FP32, tag='psB', name=f'pk_{i}')
                    pv = psb.tile([128, 1024], FP32, tag='psB', name=f'pv_{i}')
                    lhs = lambda k2: xfm8[:, 2 * k2:2 * k2 + 2,
                                          i * 128:(i + 1) * 128]
                    for k2 in range(NK2):
                        for ch in range(2):
                            csl = slice(ch * 512, (ch + 1) * 512)
                            nc.tensor.matmul(
                                pk[:, csl], lhsT=lhs(k2),
                                rhs=wk8[:, 2 * k2:2 * k2 + 2, csl],
                                start=(k2 == 0), stop=(k2 == NK2 - 1),
                                perf_mode=DR)
                    for k2 in range(NK2):
                        for ch in range(2):
                            csl = slice(ch * 512, (ch + 1) * 512)
                            nc.tensor.matmul(
                                pv[:, csl], lhsT=lhs(k2),
                                rhs=wv8[:, 2 * k2:2 * k2 + 2, csl],
                                start=(k2 == 0), stop=(k2 == NK2 - 1),
                                perf_mode=DR)
                    # k epilogue on ACT: relu(scol*pk) == scol*relu(pk)
                    # (scol, ccol >= 0)
                    for ch in range(2):
                        csl = slice(ch * 512, (ch + 1) * 512)
                        pkv = pk[:, csl].rearrange('p (h d) -> p h d', d=64)
                        nc.scalar.activation(
                            out=ksc8[:, i, ch * 8:(ch + 1) * 8, 0:64],
                            in_=pkv, func=AF.Relu, scale=scol[:, i:i + 1])
                        nc.scalar.activation(
                            out=ksc8[:, i, ch * 8:(ch + 1) * 8, 64:128],
                            in_=pkv, func=AF.Relu, scale=ccol[:, i:i + 1])
                        nc.scalar.activation(
                            out=vaug8[:, i, ch * 8:(ch + 1) * 8, 0:64],
                            in_=pv[:, csl].rearrange('p (h d) -> p h d', d=64),
                            func=AF.Copy, scale=SKV)
                    # kv accumulation for heads 0-7 interleaved per token
                    # tile. Only the very first MM per PSUM bank carries
                    # start=True (bank clear); i=0 writes then land on
                    # has_written=0 regions (overwrite), i>0 accumulate.
                    for g in range(2):
                        for hh in range(4):
                            h = 4 * g + hh
                            nc.tensor.matmul(
                                kvt[g][:, hh, :],
                                lhsT=ksc8[:, i, h, :],
                                rhs=vaug8[:, i, h, :],
                                start=(i == 0 and hh == 0),
                                stop=(i == NT - 1))
                # split AllReduce: heads 0-7 trigger first. PSUM->SBUF
                # copies on ACT (must not queue behind DVE work).
                kvv = kvp[:].rearrange('p (h c) -> p h c', c=65)
                for g in range(2):
                    nc.scalar.activation(out=kvv[:, g * 4:(g + 1) * 4, :],
                                         in_=kvt[g], func=AF.Copy)
                nc.gpsimd.dma_start(out=kv_cc_in1[:], in_=kvp[:, 0:520])
                nc.gpsimd.collective_compute(
                    'AllReduce', AluOpType.add,
                    ins=[kv_cc_in1.opt()], outs=[kv_cc_out1.opt()],
                    replica_groups=RG)
                # heads 8-15 accumulate during AR1's latency window
                kvt2 = [pskv.tile([128, 4, 65], FP32, tag='psKV',
                                  name=f'kvt2_{g}') for g in range(2)]
                for g in range(2):
                    for hh in range(4):
                        h = 8 + 4 * g + hh
                        for i in range(NT):
                            nc.tensor.matmul(
                                kvt2[g][:, hh, :],
                                lhsT=ksc8[:, i, h, :],
                                rhs=vaug8[:, i, h, :],
                                start=(i == 0 and hh == 0),
                                stop=(i == NT - 1))
                for g in range(2):
                    nc.scalar.activation(
                        out=kvv[:, (g + 2) * 4:(g + 3) * 4, :],
                        in_=kvt2[g], func=AF.Copy)
                nc.gpsimd.dma_start(out=kv_cc_in2[:], in_=kvp[:, 520:1040])
                nc.gpsimd.collective_compute(
                    'AllReduce', AluOpType.add,
                    ins=[kv_cc_in2.opt()], outs=[kv_cc_out2.opt()],
                    replica_groups=RG)

            kvb_bf = pp.tile([128, H * 65], BF, tag='kvb_bf')
            nc.gpsimd.dma_start(out=kvb_bf[:, 0:520], in_=kv_cc_out1[:])
            nc.gpsimd.dma_start(out=kvb_bf[:, 520:1040], in_=kv_cc_out2[:])

            # ================= D phase: q projection (feature-major) ========
            # pq[j,t] = (x @ Wq)[j,t] * 2^8 ; LN1 folds in via
            # t1 = pq + mu_b * (-colsum(Wq) * 2^8), then relu, then
            # qq8 = relu(t1) * (s|c * rstd * 2^-3)
            with (
                tc.tile_pool(name='psD', bufs=3, space='PSUM') as psd,
                tc.tile_pool(name='qsb', bufs=3) as qsb,
            ):
                for j in range(NJ):
                    pq = psd.tile([128, 1024], FP32, tag='psD',
                                  name=f'pq_{j}')
                    for k2 in range(NK2):
                        for ch in range(2):
                            csl = slice(ch * 512, (ch + 1) * 512)
                            nc.tensor.matmul(
                                pq[:, csl],
                                lhsT=wq8[:, 2 * k2:2 * k2 + 2,
                                         j * 128:(j + 1) * 128],
                                rhs=xfm8[:, 2 * k2:2 * k2 + 2, csl],
                                start=(k2 == 0), stop=False,
                                perf_mode=DR)
                    # rank-1 LN1 fold: pq += (-colsum(Wq) row)^T @ mu row
                    for ch in range(2):
                        csl = slice(ch * 512, (ch + 1) * 512)
                        nc.tensor.matmul(
                            pq[:, csl],
                            lhsT=cswt[0:1, j * 128:(j + 1) * 128],
                            rhs=bcT[0:1, csl], start=False, stop=True)
                    qrel = qsb.tile([128, 1024], BF, tag='qrel')
                    nc.scalar.activation(out=qrel, in_=pq, func=AF.Relu)
                    h0, h1 = 2 * j, 2 * j + 1
                    nc.vector.tensor_tensor(
                        out=qq8[0:64, h0, :], in0=qrel[0:64, :],
                        in1=sbt[0:64, :], op=AluOpType.mult)
                    nc.vector.tensor_tensor(
                        out=qq8[64:128, h0, :], in0=qrel[0:64, :],
                        in1=cbt[0:64, :], op=AluOpType.mult)
                    nc.vector.tensor_tensor(
                        out=qq8[0:64, h1, :], in0=qrel[64:128, :],
                        in1=sbt[64:128, :], op=AluOpType.mult)
                    nc.vector.tensor_tensor(
                        out=qq8[64:128, h1, :], in0=qrel[64:128, :],
                        in1=cbt[64:128, :], op=AluOpType.mult)

            # ================= E phase: attention + LN2 (token-major) =======
            xhT = pp.tile([128, NJ, 512], BF, tag='xhT')

            def emit_xh_transpose(tsl, par):
                for j in range(NJ):
                    eng = nc.sync if j % 2 == par else nc.scalar
                    eng.dma_start(out=xhT[:, j, :],
                                  in_=xh_dram[tsl, j * 128:(j + 1) * 128],
                                  transpose=True)
                    nc.scalar.activation(out=xhT8[:, j, tsl],
                                         in_=xhT[:, j, :], func=AF.Copy)

            with (
                tc.tile_pool(name='psE', bufs=6, space='PSUM') as pse,
                tc.tile_pool(name='ef', bufs=3) as efp,
                tc.tile_pool(name='psG', bufs=2, space='PSUM') as psg,
                tc.tile_pool(name='go', bufs=3) as gop,
            ):
                def emit_attn_pass(i, p):
                    rsl = slice(i * 128, (i + 1) * 128)
                    z8 = efp.tile([128, 8], FP32, tag='z8',
                                  name=f'z8_{i}_{p}')
                    pas = []
                    for gg in range(2):
                        g = 2 * p + gg
                        pa = pse.tile([128, 4, 65], FP32, tag='psE',
                                      name=f'pa_{i}_{g}')
                        pas.append(pa)
                        for hh in range(4):
                            h = 4 * g + hh
                            nc.tensor.matmul(
                                pa[:, hh, :], lhsT=qq8[:, h, rsl],
                                rhs=kvb_bf[:, h * 65:(h + 1) * 65],
                                start=True, stop=True)
                        nc.vector.reciprocal(out=z8[:, gg * 4:(gg + 1) * 4],
                                             in_=pa[:, :, 64])
                    for gg in range(2):
                        g = 2 * p + gg
                        zb = z8[:, gg * 4:(gg + 1) * 4].broadcast_to(
                            (128, 4, 64))
                        nc.vector.tensor_tensor(
                            out=yt_all[:, i, g * 4:(g + 1) * 4, :],
                            in0=pas[gg][:, :, 0:64], in1=zb,
                            op=AluOpType.mult)
                    hsl = slice(p * 512, (p + 1) * 512)
                    ytr = yt_all[:, i, :, :].rearrange('p h d -> p (h d)')
                    nc.vector.tensor_tensor(out=ytr[:, hsl], in0=ytr[:, hsl],
                                            in1=qn_sb[:, i, hsl],
                                            op=AluOpType.add)

                def emit_ln2(i):
                    rsl = slice(i * 128, (i + 1) * 128)
                    ytf = yt_all[:, i, :, :].rearrange('p h d -> p (h d)')
                    asum = efp.tile([128, 1], FP32, tag='easum')
                    sqs = efp.tile([128, 1], FP32, tag='esqs')
                    nc.scalar.activation(out=ascr, in_=ytf, func=AF.Copy,
                                         accum_out=asum)
                    nc.scalar.activation(out=ascr, in_=ytf, func=AF.Square,
                                         accum_out=sqs)
                    mu2 = efp.tile([128, 1], FP32, tag='emu')
                    nc.vector.tensor_scalar_mul(out=mu2, in0=asum,
                                                scalar1=INV_E)
                    m2 = efp.tile([128, 1], FP32, tag='em2')
                    nc.vector.tensor_tensor(out=m2, in0=mu2, in1=mu2,
                                            op=AluOpType.mult)
                    var = efp.tile([128, 1], FP32, tag='evar')
                    nc.vector.tensor_scalar(out=var, in0=sqs, scalar1=INV_E,
                                            scalar2=m2,
                                            op0=AluOpType.mult,
                                            op1=AluOpType.subtract)
                    sd = efp.tile([128, 1], FP32, tag='esd')
                    nc.scalar.activation(out=sd, in_=var, func=AF.Sqrt,
                                         bias=eps1, scale=1.0)
                    rstd2 = efp.tile([128, 1], FP32, tag='erstd')
                    nc.vector.reciprocal(out=rstd2, in_=sd)
                    nc.vector.tensor_scalar(out=xh_sb[:, i, :], in0=ytf,
                                            scalar1=mu2, scalar2=rstd2,
                                            op0=AluOpType.subtract,
                                            op1=AluOpType.mult)
                    nc.gpsimd.dma_start(
                        out=xh_dram[rsl, :], in_=xh_sb[:, i, :])

                def emit_o(i):
                    for ch in range(2):
                        csl = slice(ch * 512, (ch + 1) * 512)
                        po = psg.tile([128, 512], FP32, tag='psG',
                                      name=f'po_{i}_{ch}')
                        for k2 in range(NK2):
                            nc.tensor.matmul(
                                po,
                                lhsT=xhT8[:, 2 * k2:2 * k2 + 2,
                                          i * 128:(i + 1) * 128],
                                rhs=wo8[:, 2 * k2:2 * k2 + 2, csl],
                                start=(k2 == 0), stop=(k2 == NK2 - 1),
                                perf_mode=DR)
                        ot2 = gop.tile([128, 512], BF, tag='ot2')
                        nc.scalar.activation(out=ot2, in_=po, func=AF.Copy,
                                             scale=SO)
                        ot = gop.tile([128, 512], FP32, tag='ot')
                        nc.vector.tensor_tensor(
                            out=ot, in0=ot2, in1=xh_sb[:, i, csl],
                            op=AluOpType.add)
                        nc.sync.dma_start(
                            out=d_out[i * 128:(i + 1) * 128, csl], in_=ot)

                for i in range(NT):
                    emit_attn_pass(i, 0)
                for i in range(4):
                    emit_attn_pass(i, 1)
                    emit_ln2(i)
                emit_xh_transpose(slice(0, 512), 0)
                for i in range(4, NT):
                    emit_attn_pass(i, 1)
                    emit_ln2(i)
                    emit_o(i - 4)
                emit_xh_transpose(slice(512, 1024), 1)
                for i in range(4, NT):
                    emit_o(i)

    nc.compile()
    return nc


def _get_program():
    if 'p' not in _BUILD_CACHE:
        _BUILD_CACHE['p'] = _build_program()
    return _BUILD_CACHE['p']


def _phm_weight(A, S):
    f = A.shape[0]
    din, dout = f * S.shape[1], f * S.shape[2]
    W = np.einsum('nij,nkl->ikjl', np.asarray(A, np.float32),
                  np.asarray(S, np.float32))
    return np.ascontiguousarray(W.reshape(din, dout))


def _reference_np(query, qA, qS, qb, kA, kS, kb, vA, vS, vb, oA, oS, ob,
                  g1, b1, g2, b2):
    """Pure-numpy fallback (only used for non-default affine/bias inputs)."""
    x = np.asarray(query, np.float64)
    Lq, Nq, Eq = x.shape

    def ln(t, g, b, eps=1e-5):
        mu = t.mean(-1, keepdims=True)
        var = ((t - mu) ** 2).mean(-1, keepdims=True)
        return (t - mu) / np.sqrt(var + eps) * g + b

    def phm(t, A, S, bb):
        W = np.einsum('nij,nkl->ikjl', np.asarray(A, np.float64),
                      np.asarray(S, np.float64)).reshape(Eq, Eq)
        return t @ W + np.asarray(bb, np.float64)

    qn = ln(x, g1, b1)
    q = np.maximum(phm(qn, qA, qS, qb), 0)
    k = np.maximum(phm(x, kA, kS, kb), 0)
    v = phm(x, vA, vS, vb)
    resh = lambda t: t.reshape(Lq, Nq * H, D).transpose(1, 0, 2)
    q, k, v = resh(q), resh(k), resh(v)
    idx = (np.pi / 2) * np.arange(1, Lq + 1)[None, :, None] / Lq
    s, c = np.sin(idx), np.cos(idx)
    q_ = np.concatenate([q * s, q * c], -1)
    k_ = np.concatenate([k * s, k * c], -1)
    kv = np.einsum('nld,nlm->ndm', k_, v)
    z = 1.0 / np.maximum(np.einsum('nld,nd->nl', q_, k_.sum(1)), 1e-6)
    attn = np.einsum('nld,ndm->nlm', q_, kv) * z[..., None]
    attn = attn.transpose(1, 0, 2).reshape(Lq, Nq, Eq) + qn
    out = ln(attn, g2, b2)
    return (phm(out, oA, oS, ob) + out).astype(np.float32)


def kernel(**inputs):
    query = np.asarray(inputs['query'], np.float32)
    g1 = np.asarray(inputs['g1'], np.float32)
    b1 = np.asarray(inputs['b1'], np.float32)
    g2 = np.asarray(inputs['g2'], np.float32)
    b2 = np.asarray(inputs['b2'], np.float32)
    biases = [np.asarray(inputs[k], np.float32) for k in
              ('qb', 'kb', 'vb', 'ob')]

    default_affine = (np.all(g1 == 1.0) and np.all(b1 == 0.0)
                      and np.all(g2 == 1.0) and np.all(b2 == 0.0)
                      and all(not np.any(b) for b in biases))
    if not default_affine:
        return _reference_np(**inputs)

    Wq = _phm_weight(inputs['qA'], inputs['qS'])
    Wk = _phm_weight(inputs['kA'], inputs['kS'])
    Wv = _phm_weight(inputs['vA'], inputs['vS'])
    Wo = _phm_weight(inputs['oA'], inputs['oS'])
    sc = float(2.0 ** EW)
    for W in (Wq, Wk, Wv, Wo):
        assert np.abs(W).max() * sc < 240.0, "fp8 weight scale overflow"
    wq8 = (Wq * sc).astype(F8)
    wk8 = (Wk * sc).astype(F8)
    wv8 = (Wv * sc).astype(F8)
    wo8 = (Wo * sc).astype(F8)
    # -colsum(Wq) * 2^EW as a single row (partition 0)
    ncsw = (-Wq.sum(axis=0) * sc).astype(BF16).reshape(1, E)

    nc = _get_program()

    s_full = np.sin((np.pi / 2) * np.arange(1, L + 1, dtype=np.float32) / L)
    c_full = np.cos((np.pi / 2) * np.arange(1, L + 1, dtype=np.float32) / L)

    in_maps = []
    for core in range(NCORES):
        b = core // 2
        l0 = (core % 2) * T
        x = np.ascontiguousarray(query[l0:l0 + T, b, :])
        s = s_full[l0:l0 + T]
        c = c_full[l0:l0 + T]
        im = {
            'x_tm': x.astype(BF16),
            'x_fm': np.ascontiguousarray(x.T).astype(F8),
            'wq': wq8, 'wk': wk8, 'wv': wv8, 'wo': wo8,
            's_bcast': np.ascontiguousarray(
                np.broadcast_to(s * SQH, (128, T))).astype(BF16),
            'c_bcast': np.ascontiguousarray(
                np.broadcast_to(c * SQH, (128, T))).astype(BF16),
            's_cols': np.ascontiguousarray(s.reshape(NT, 128).T) * SKV,
            'c_cols': np.ascontiguousarray(c.reshape(NT, 128).T) * SKV,
            'neg_csw': ncsw,
        }
        in_maps.append(im)

    trace = bool(os.environ.get('KERNEL_TRACE'))
    kw = {}
    tdir = os.environ.get('KERNEL_TRACE_DIR')
    if trace and tdir:
        os.makedirs(tdir, exist_ok=True)
        kw['tmpdir'] = tdir
    res = run_bass_kernel_spmd(nc, in_maps, list(range(NCORES)), trace=trace,
                               **kw)
    kernel._last_exec_ns = res.exec_time_ns
    kernel._last_res = res

    out = np.empty((L, N, E), np.float32)
    for core in range(NCORES):
        b = core // 2
        l0 = (core % 2) * T
        out[l0:l0 + T, b, :] = res.results[core]['out']
    return out


kernel._last_exec_ns = None
